# revision 1
# baseline (speedup 1.0000x reference)
"""KAN B-spline activation kernel for Trainium2 (8 NeuronCores, data-parallel on batch).

Math (validated vs reference to ~1e-7 rel):
  grid is uniform: g[t] = -1 + (t-3)*h, h = 0.125, t = 0..22; u = (x - g[0])/h = 8x + 11.
  For x in [0,1) only knot-window t in [8,18] has nonzero cubic bases.
  Let A[k] = x - g[8+k], k = 0..15 (k=15 unused pad).
  B1[m]  = Relu(1 - |A[m+1]|/h)                     (hat; == Cox-de Boor level 1), m=0..12
  B2d[m] = A[m]*B1[m]   - A[m+3]*B1[m+1]           (= 2h * B2), m=0..11
  B3d[m] = A[m]*B2d[m]  - A[m+4]*B2d[m+1]          (= 6h^2 * B3), m=0..10
  out[b,o,i] = sum_m B3d[b,i,m] * coef[o,i,8+m] / (6 h^2)

Device layout (per core, 128 batch rows in partitions):
  A/B* tiles: (128, 64 inputs x 16-knot-window blocks) in the free dim.
  B3 (128, 64*16) -> 8 PE transposes of 128-col groups (8 inputs each) ->
  basesT (K=(input,knot) partitions, batch free). Matmul per (group g, subgroup s):
  K=32 (2 inputs x 16 knots), N=128 (2 inputs x 64 outs), block-diagonal rhs built
  on host with the zeros/padding baked in. PSUM bank per group (128,512) is copied
  verbatim to SBUF and DMA'd out; host un-permutes (b, g, s, p, o) -> (b, o, i).
"""

import numpy as np
from contextlib import ExitStack

import concourse.bass as bass
import concourse.tile as tile
from concourse import bacc, mybir
from concourse.bass_utils import run_bass_kernel_spmd
from concourse.masks import make_identity

N_CORES = 8
B_TOT, IN_DIM, OUT_DIM = 1024, 64, 64
BPC = B_TOT // N_CORES          # 128 batch rows per core
K16 = 16                        # padded knot window per input
NG = 8                          # groups of 8 inputs
F32 = mybir.dt.float32

# If the stride-0 broadcast read on DVE fails, flip to False (log-doubling copies).
# HW faulted with stride-0 input APs on DVE (sim accepts them); use doubling.
USE_STRIDE0 = False

_CACHE = {}


def _build_nc():
    nc = bacc.Bacc("TRN2", target_bir_lowering=False, debug=False,
                   num_devices=N_CORES)
    x_d = nc.dram_tensor("x_in", [BPC, IN_DIM], F32, kind="ExternalInput").ap()
    rhs_d = nc.dram_tensor("rhs_in", [128, NG * 512], F32, kind="ExternalInput").ap()
    g3_d = nc.dram_tensor("g3_in", [1, IN_DIM * K16], F32, kind="ExternalInput").ap()
    out_d = nc.dram_tensor("out", [BPC, NG, 512], F32, kind="ExternalOutput").ap()

    with tile.TileContext(nc) as tc, ExitStack() as ctx:
        pool = ctx.enter_context(tc.tile_pool(name="main", bufs=1))
        psT = ctx.enter_context(tc.tile_pool(name="psT", bufs=2, space="PSUM"))
        psO = ctx.enter_context(tc.tile_pool(name="psO", bufs=4, space="PSUM"))
        og_pool = ctx.enter_context(tc.tile_pool(name="og", bufs=4))

        ident = pool.tile([128, 128], F32)
        make_identity(nc, ident)

        x_sb = pool.tile([BPC, IN_DIM], F32)
        nc.sync.dma_start(out=x_sb[:], in_=x_d)
        rhs_sb = pool.tile([128, NG * 512], F32)
        nc.sync.dma_start(out=rhs_sb[:], in_=rhs_d)
        # broadcast the (1, 1024) knot row across 128 partitions during DMA
        g3_sb = pool.tile([128, IN_DIM * K16], F32)
        g3_bcast = bass.AP(tensor=g3_d.tensor, offset=g3_d.offset,
                           ap=[[0, 128]] + list(g3_d.ap[1:]))
        nc.gpsimd.dma_start(out=g3_sb[:], in_=g3_bcast)
        g3v = g3_sb[:].rearrange("p (i k) -> p i k", k=K16)

        # broadcast x along the 16-knot window by log-doubling copies
        xt = pool.tile([BPC, IN_DIM, K16], F32)
        nc.vector.tensor_copy(xt[:, :, 0:1],
                              x_sb[:].rearrange("p (i k) -> p i k", k=1))
        w = 1
        while w < K16:
            n = min(w, K16 - w)
            nc.vector.tensor_copy(xt[:, :, w:w + n], xt[:, :, 0:n])
            w += n

        halves = ctx.enter_context(tc.tile_pool(name="halves", bufs=2))
        basesT = pool.tile([128, NG * 128], F32)
        HW_IN = IN_DIM // 2                       # 32 inputs per half
        for H in range(2):
            isl = slice(H * HW_IN, (H + 1) * HW_IN)
            Ah = halves.tile([BPC, HW_IN, K16], F32)
            nc.vector.tensor_sub(Ah[:], xt[:, isl, :], g3v[:, isl, :])
            Bab = halves.tile([BPC, HW_IN, 13], F32)
            nc.scalar.activation(out=Bab[:], in_=Ah[:, :, 1:14],
                                 func=mybir.ActivationFunctionType.Abs)
            B1h = halves.tile([BPC, HW_IN, 13], F32)
            # Relu(-8*|A| + 1) == Relu(1 - |A|/h)
            nc.scalar.activation(out=B1h[:], in_=Bab[:],
                                 func=mybir.ActivationFunctionType.Relu,
                                 scale=-8.0, bias=1.0)
            Ml2 = halves.tile([BPC, HW_IN, 12], F32)
            Mr2 = halves.tile([BPC, HW_IN, 12], F32)
            B2h = halves.tile([BPC, HW_IN, 12], F32)
            nc.vector.tensor_mul(Ml2[:], Ah[:, :, 0:12], B1h[:, :, 0:12])
            nc.vector.tensor_mul(Mr2[:], Ah[:, :, 3:15], B1h[:, :, 1:13])
            nc.vector.tensor_sub(B2h[:], Ml2[:], Mr2[:])
            Ml3 = halves.tile([BPC, HW_IN, 11], F32)
            Mr3 = halves.tile([BPC, HW_IN, 11], F32)
            B3h = halves.tile([BPC, HW_IN, K16], F32)
            nc.vector.tensor_mul(Ml3[:], Ah[:, :, 0:11], B2h[:, :, 0:11])
            nc.vector.tensor_mul(Mr3[:], Ah[:, :, 4:15], B2h[:, :, 1:12])
            # pad knots 11..15 must be 0: they feed the transpose, whose
            # output multiplies real coef columns.
            nc.vector.memset(B3h[:, :, 11:16], 0.0)
            nc.vector.tensor_sub(B3h[:, :, 0:11], Ml3[:], Mr3[:])

            B3f = B3h[:].rearrange("p i k -> p (i k)")
            ps_t = psT.tile([128, 512], F32)
            for q in range(4):
                nc.tensor.transpose(out=ps_t[:, q * 128:(q + 1) * 128],
                                    in_=B3f[:, q * 128:(q + 1) * 128],
                                    identity=ident[:])
            dst = basesT[:, H * 512:(H + 1) * 512]
            if H == 0:
                nc.vector.tensor_copy(dst, ps_t[:])
            else:
                nc.scalar.copy(dst, ps_t[:])

            for q in range(4):
                g = 4 * H + q
                ps_o = psO.tile([128, 512], F32)
                nc.tensor.matmul(out=ps_o[:],
                                 lhsT=basesT[:, g * 128:(g + 1) * 128],
                                 rhs=rhs_sb[:, g * 512:(g + 1) * 512],
                                 start=True, stop=True)
                og = og_pool.tile([128, 512], F32)
                if g % 2 == 0:
                    nc.vector.tensor_copy(og[:], ps_o[:])
                else:
                    nc.scalar.copy(og[:], ps_o[:])
                nc.sync.dma_start(out=out_d[:, g, :], in_=og[:])

    nc.compile()
    return nc


def _host_inputs(x, coef, grid):
    x = np.ascontiguousarray(np.asarray(x, dtype=np.float32))
    coef = np.asarray(coef, dtype=np.float32)
    knots = np.asarray(grid, dtype=np.float32)[0, 0, :]          # (23,)
    h = float(knots[1] - knots[0])

    g3 = np.empty(K16, dtype=np.float32)
    g3[:15] = knots[8:23]
    g3[15] = knots[22] + h                                       # unused pad
    g3row = np.tile(g3, IN_DIM)[None, :]                         # (1, 1024)

    scale = 1.0 / (6.0 * h * h)
    cf = coef[:, :, 8:19] * scale                                # (o, i, 11)
    # block-diagonal rhs per group: rows (i_l,j) x cols (i_l', o), K=128, N=512
    rhs = np.zeros((128, NG * 512), dtype=np.float32)
    for i_l in range(8):
        for g in range(NG):
            i = g * 8 + i_l
            rhs[i_l * 16:i_l * 16 + 11,
                g * 512 + i_l * 64:g * 512 + i_l * 64 + 64] = cf[:, i, :].T
    return x, rhs, g3row


def _execute(x, coef, grid, trace=False, **spmd_kwargs):
    xf, rhs, g3row = _host_inputs(x, coef, grid)
    if "nc" not in _CACHE:
        _CACHE["nc"] = _build_nc()
    nc = _CACHE["nc"]
    in_maps = [{"x_in": np.ascontiguousarray(xf[c * BPC:(c + 1) * BPC]),
                "rhs_in": rhs, "g3_in": g3row} for c in range(N_CORES)]
    res = run_bass_kernel_spmd(nc, in_maps, list(range(N_CORES)),
                               trace=trace, **spmd_kwargs)
    full = np.empty((B_TOT, OUT_DIM, IN_DIM), dtype=np.float32)
    for c in range(N_CORES):
        t = res.results[c]["out"].reshape(BPC, NG, 8, 64)        # (b, g, i_l, o)
        full[c * BPC:(c + 1) * BPC] = (
            t.transpose(0, 3, 1, 2).reshape(BPC, OUT_DIM, IN_DIM))
    return full, res


def kernel(x, coef, grid):
    out, _ = _execute(x, coef, grid, trace=False)
    return out



# revision 2
# speedup vs baseline: 1.0005x; 1.0005x over previous
"""KAN B-spline activation kernel for Trainium2 (8 NeuronCores, data-parallel on batch).

Math (u-space, validated vs reference; bf16 chain gives ~5e-3 rel):
  grid is uniform: h = 0.125, u = (x - knots[8])/h = 8x + 3; u in [3, 11) for x in [0,1).
  Au[k] = u - k, k = 0..14 (f32, log-doubled on scalar; cast once to bf16).
  B1[m]  = Relu(1 - |Au[m+1]|), m = 0..12                  (hat, scalar, bf16 out)
  B2[m]  = Au[m]*(B1[m]-B1[m+1]) + 3*B1[m+1], m = 0..11   (since Au[m+3] = Au[m]-3)
  B3[m]  = Au[m]*(B2[m]-B2[m+1]) + 4*B2[m+1], m = 0..10
  out[b,o,i] = sum_m B3[b,i,m] * coef[o,i,8+m] / 6

Per core (128 batch rows in partitions):
  x DMA issued first on the scalar ring, rhs on the gpsimd ring (keeps x's
  descriptors ahead of the 1MB rhs load). Level-2/3 chains run bf16 on vector
  only (concurrent vector+gpsimd elementwise halves both engines' rates).
  B3 bf16 -> 8 PE transposes (fast weight load) -> basesT bf16. Matmul per
  group g (8 inputs): K=128, N=512 vs bf16 block-diagonal rhs from host
  (coef/6, zeros baked). PSUM f32 -> bf16 cast-copies alternating
  scalar/vector -> output DMA per group pair (2KB rows) on sync (pairs 0,1)
  and gpsimd (pairs 2,3) rings. Host un-permutes (b, g, i_l, o) -> (b, o, i).
"""

import numpy as np
from contextlib import ExitStack

import ml_dtypes
import concourse.bass as bass
import concourse.tile as tile
from concourse import bacc, mybir
from concourse.bass_utils import run_bass_kernel_spmd
from concourse.masks import make_identity

N_CORES = 8
B_TOT, IN_DIM, OUT_DIM = 1024, 64, 64
BPC = B_TOT // N_CORES          # 128 batch rows per core
K16 = 16                        # padded knot window per input (transpose block)
KA = 15                         # Au window
NG = 8                          # groups of 8 inputs
F32 = mybir.dt.float32
BF16 = mybir.dt.bfloat16
AF = mybir.ActivationFunctionType
ALU = mybir.AluOpType

_CACHE = {}


def _build_nc(u_scale, u_bias):
    """u = u_scale*x + u_bias maps x onto integer-knot spline coordinates."""
    nc = bacc.Bacc("TRN2", target_bir_lowering=False, debug=False,
                   num_devices=N_CORES)
    x_d = nc.dram_tensor("x_in", [BPC, IN_DIM], F32, kind="ExternalInput").ap()
    rhs_d = nc.dram_tensor("rhs_in", [128, NG * 512], BF16,
                           kind="ExternalInput").ap()
    out_d = nc.dram_tensor("out", [BPC, 4, 1024], BF16,
                           kind="ExternalOutput").ap()

    with tile.TileContext(nc) as tc, ExitStack() as ctx:
        pool = ctx.enter_context(tc.tile_pool(name="main", bufs=1))
        psT = ctx.enter_context(tc.tile_pool(name="psT", bufs=2, space="PSUM"))
        psO = ctx.enter_context(tc.tile_pool(name="psO", bufs=6, space="PSUM"))
        og_pool = ctx.enter_context(tc.tile_pool(name="og", bufs=4))
        halves = ctx.enter_context(tc.tile_pool(name="halves", bufs=2))

        x_sb = pool.tile([BPC, IN_DIM], F32)
        nc.scalar.dma_start(out=x_sb[:], in_=x_d)

        # identity before the rhs DMA so x's descriptors own the queues first;
        # rhs still lands well before the first matmul needs it
        ident = pool.tile([128, 128], BF16)
        make_identity(nc, ident)
        rhs_sb = pool.tile([128, NG * 512], BF16)
        nc.gpsimd.dma_start(out=rhs_sb[:], in_=rhs_d)
        x3 = x_sb[:].rearrange("p (i k) -> p i k", k=1)          # (128, 64, 1)

        # Au[:, i, k] = u - k via log-doubling on vector (f32), one bf16 cast
        Au = pool.tile([BPC, IN_DIM, KA], F32)
        nc.vector.tensor_scalar(out=Au[:, :, 0:1], in0=x3,
                                scalar1=u_scale, scalar2=u_bias,
                                op0=ALU.mult, op1=ALU.add)
        for d, n in ((1, 1), (2, 2), (4, 4), (8, 7)):
            nc.vector.tensor_scalar_sub(Au[:, :, d:d + n], Au[:, :, 0:n],
                                        float(d))
        Aub = pool.tile([BPC, IN_DIM, KA], BF16)
        nc.vector.tensor_copy(Aub[:], Au[:])

        B1 = pool.tile([BPC, IN_DIM, 13], BF16)
        B3 = pool.tile([BPC, IN_DIM, K16], BF16)
        nc.vector.memset(B3[:, :, 11:16], 0.0)
        basesT = pool.tile([128, NG * 128], BF16)

        # hat per half on scalar: Abs then Relu, bf16 out
        Bab = pool.tile([BPC, IN_DIM, 13], F32)
        HW_IN = IN_DIM // 2                       # 32 inputs per half
        for H in range(2):
            isl = slice(H * HW_IN, (H + 1) * HW_IN)
            nc.scalar.activation(out=Bab[:, isl, :], in_=Au[:, isl, 1:14],
                                 func=AF.Abs)
            nc.scalar.activation(out=B1[:, isl, :], in_=Bab[:, isl, :],
                                 func=AF.Relu, scale=-1.0, bias=1.0)

        # vector: levels 2 and 3 per half, bf16, fused difference form; the
        # final level-3 op is split per quarter (16 inputs = 2 matmul groups)
        # so transposes/matmuls start while the chain is still running
        for H in range(2):
            isl = slice(H * HW_IN, (H + 1) * HW_IN)
            D1 = halves.tile([BPC, HW_IN, 12], BF16)
            P2 = halves.tile([BPC, HW_IN, 12], BF16)
            B2 = halves.tile([BPC, HW_IN, 12], BF16)
            nc.vector.tensor_sub(D1[:], B1[:, isl, 0:12], B1[:, isl, 1:13])
            nc.vector.tensor_mul(P2[:], Aub[:, isl, 0:12], D1[:])
            nc.vector.scalar_tensor_tensor(out=B2[:], in0=B1[:, isl, 1:13],
                                           scalar=3.0, in1=P2[:],
                                           op0=ALU.mult, op1=ALU.add)
            D2 = halves.tile([BPC, HW_IN, 11], BF16)
            P3 = halves.tile([BPC, HW_IN, 11], BF16)
            nc.vector.tensor_sub(D2[:], B2[:, :, 0:11], B2[:, :, 1:12])
            nc.vector.tensor_mul(P3[:], Aub[:, isl, 0:11], D2[:])
            for j in range(2):                    # quarter qq = 2*H + j
                qsl = slice(H * HW_IN + j * 16, H * HW_IN + (j + 1) * 16)
                lsl = slice(j * 16, (j + 1) * 16)
                nc.vector.scalar_tensor_tensor(
                    out=B3[:, qsl, 0:11], in0=B2[:, lsl, 1:12], scalar=4.0,
                    in1=P3[:, lsl, :], op0=ALU.mult, op1=ALU.add)

        # per quarter: 2 transposes -> basesT copy -> 2 single-bank matmuls
        # (6 PSUM bufs so no matmul ever waits on a cast draining its bank)
        # -> single casts on alternating engines -> pair DMA.
        copy_eng = [nc.scalar, nc.scalar, nc.vector, nc.vector]
        for qq in range(4):
            isl = slice(qq * 16, (qq + 1) * 16)
            B3f = B3[:, isl, :].rearrange("p i k -> p (i k)")
            ps_t = psT.tile([128, 256], BF16)
            for r in range(2):
                nc.tensor.transpose(out=ps_t[:, r * 128:(r + 1) * 128],
                                    in_=B3f[:, r * 128:(r + 1) * 128],
                                    identity=ident[:])
            dst = basesT[:, qq * 256:(qq + 1) * 256]
            eng = copy_eng[qq]
            if eng is nc.scalar:
                eng.copy(dst, ps_t[:])
            else:
                eng.tensor_copy(dst, ps_t[:])

            og = og_pool.tile([128, 2, 512], BF16)
            for r in range(2):
                g = 2 * qq + r
                ps_o = psO.tile([128, 512], F32)
                nc.tensor.matmul(out=ps_o[:],
                                 lhsT=basesT[:, g * 128:(g + 1) * 128],
                                 rhs=rhs_sb[:, g * 512:(g + 1) * 512],
                                 start=True, stop=True)
                if r == 0:
                    nc.scalar.copy(og[:, 0, :], ps_o[:])
                else:
                    nc.vector.tensor_copy(og[:, 1, :], ps_o[:])
            ring = nc.sync if qq % 2 == 0 else nc.gpsimd
            ring.dma_start(out=out_d[:, qq, :],
                           in_=og[:].rearrange("p a b -> p (a b)"))

    nc.compile()
    return nc


def _host_inputs(x, coef, grid):
    x = np.ascontiguousarray(np.asarray(x, dtype=np.float32))
    coef = np.asarray(coef, dtype=np.float32)
    knots = np.asarray(grid, dtype=np.float32)[0, 0, :]          # (23,)
    h = float(knots[1] - knots[0])
    u_scale = 1.0 / h
    u_bias = -float(knots[8]) / h

    cf = coef[:, :, 8:19] * (1.0 / 6.0)                          # (o, i, 11)
    # block-diagonal rhs per group: rows (i_l, j) x cols (i_l', o), K=128, N=512
    rhs = np.zeros((128, NG * 512), dtype=np.float32)
    for i_l in range(8):
        for g in range(NG):
            i = g * 8 + i_l
            rhs[i_l * 16:i_l * 16 + 11,
                g * 512 + i_l * 64:g * 512 + i_l * 64 + 64] = cf[:, i, :].T
    return x, rhs.astype(ml_dtypes.bfloat16), u_scale, u_bias


def _execute(x, coef, grid, trace=False, **spmd_kwargs):
    xf, rhs, u_scale, u_bias = _host_inputs(x, coef, grid)
    if "nc" not in _CACHE:
        _CACHE["nc"] = _build_nc(u_scale, u_bias)
    nc = _CACHE["nc"]
    in_maps = [{"x_in": np.ascontiguousarray(xf[c * BPC:(c + 1) * BPC]),
                "rhs_in": rhs} for c in range(N_CORES)]
    res = run_bass_kernel_spmd(nc, in_maps, list(range(N_CORES)),
                               trace=trace, **spmd_kwargs)
    full = np.empty((B_TOT, OUT_DIM, IN_DIM), dtype=np.float32)
    for c in range(N_CORES):
        t = np.asarray(res.results[c]["out"]).astype(np.float32)
        t = t.reshape(BPC, NG, 8, 64)                            # (b, g, i_l, o)
        full[c * BPC:(c + 1) * BPC] = (
            t.transpose(0, 3, 1, 2).reshape(BPC, OUT_DIM, IN_DIM))
    return full, res


def kernel(x, coef, grid):
    out, _ = _execute(x, coef, grid, trace=False)
    return out
